# revision 7
# baseline (speedup 1.0000x reference)
"""DenseCL contrastive-logits kernel for 8 Trainium2 NeuronCores.

Contract: kernel(**inputs) takes the FULL unsharded inputs (named as in
setup_inputs) and returns the full [32, 65537, 50] float32 output.

Sharding: the 65536-wide negative queues are split along the queue axis
across the 8 cores (8192 columns each); every other input is replicated.
There are NO collectives: profiling showed the runtime's cross-core sync
barrier + ncfw latency puts a ~70-95 us floor on any collective-gated
work, so instead EVERY core redundantly computes the match/gather stage
(cosine + argmax + d_q gather) for all 32 batches from fp16 features
(12.8 MB/core).  The feature DMA overlaps the cosine accumulation
matmuls chunk by chunk, then phase 2 (the 25.7 MB out_d stream) runs
DMA-bound with no cross-core dependency at all.

Precision: the match cosine runs in fp16.  Verified on the generated
inputs: the post-fp16-rounding top-2 margin of the cosine (0.0059 worst
case) is ~60x the fp32 accumulation noise, and the fp16 input rounding
is deterministic and identical between numpy and the PE, so the argmax
reproduces the reference's fp32 choice exactly.  The negative-logit
matmuls and outputs run in fp16 (values are O(50); ~4e-4 relative
error): single-PE-pass matmuls and half the output DMA bytes.  fp16
subnormals are flushed on the host (the PE weight path mishandles them).

Math (per batch b, t = 1/tau = 5 folded into the one-hot):
  cosT[j, i] = sum_c feat_q[b, c, j] * feat_k[b, c, i]     (PE fp16,
               8 batches packed per PSUM bank: 4 col slots x 2 halves)
  onehotT[j, i] = t * (cosT[j, i] >= max_i cosT[j, :])      (DVE)
  onehot = onehotT^T                                        (PE transpose)
  d_qm5[d, j] = sum_i d_qT[b, i, d] * onehot[i, j]          (PE fp16)
  out_d[q, b, s] = sum_d queue_d[d, q] * d_qm5[b, d, s]     (PE fp16, q-shard)
  out_g[b, q]   = sum_d t * g_q[b, d] * queue_g[d, q]       (PE fp16, q-shard)
  pos_d[b, s]   = sum_d d_k[b, d, s] * d_qm5[b, d, s]       (all b)
  pos_g[b]      = t * sum_d g_q[b, d] * g_k[b, d]           (all b)
"""

import numpy as np

BS, DIM, S, CF, Q = 32, 128, 49, 2048, 65536
NCORES = 8
QS = Q // NCORES          # 8192 queue columns per core
BG = 4                    # batch groups in the big matmul
BPG = BS // BG            # 8 batches per group (8*49 = 392 fp32 < 1 psum bank)
CT = CF // 128            # 16 contraction chunks for the cosine
QT = QS // 128            # 64 queue tiles per core
INV_TAU = 5.0

_CACHE = {}


def _install_tile_drain_patch():
    """walrus in this container rejects instructions with >1 sync wait
    ("Too many sync wait commands" in setupSyncWait).  TileContext's
    end-of-kernel drain carries one wait per semaphore used; split them
    across a chain of single-wait drain instructions (same engine, same
    semantics)."""
    import concourse.tile as tile_mod
    import concourse.mybir as mybir
    from concourse.vector_clock import ScopedClock

    if getattr(tile_mod.TileContext, "_drain_patch_installed", False):
        return

    def _drain_and_barrier(self, tick_clock, wait_clock):
        nc = self.nc
        drain_inst = nc.sync.drain()
        wait_clock.add_sem_waits(
            drain_inst.ins, ScopedClock({None: tick_clock.global_clock})
        )
        waits = list(drain_inst.ins.sync_info.on_wait)
        if len(waits) > 1:
            drain_inst.ins.sync_info = mybir.SyncInfo(
                on_wait=waits[:1], on_update=[]
            )
            for i in range(1, len(waits)):
                extra = nc.sync.drain()
                extra.ins.sync_info = mybir.SyncInfo(
                    on_wait=waits[i : i + 1], on_update=[]
                )
        nc.all_engine_barrier()
        assert self.sems is not None
        popped = nc._tile_sem_poison_stack.pop()
        assert popped is self._sem_poison
        nc.clear_and_free_semaphores(list(self.sems.allocated().values()))
        nc.all_engine_barrier()

    tile_mod.TileContext._drain_and_barrier = _drain_and_barrier
    tile_mod.TileContext._drain_patch_installed = True


def _split_multi_waits(nc, mybir, limit=1):
    """walrus codegen here rejects instructions with more than one sync
    wait.  Hoist excess waits onto InstNoOp carriers inserted immediately
    before the offender in the same block (same engine stream => same
    semantics: all waits still execute before the instruction)."""
    n_new = 0
    for f in nc.m.functions:
        for bb in f.blocks:
            new_list = []
            changed = False
            for inst in bb.instructions:
                si = inst.sync_info
                waits = list(si.on_wait) if si is not None else []
                if len(waits) > limit:
                    for w in waits[limit:]:
                        n_new += 1
                        nop = mybir.InstNoOp(name=f"WS-{n_new}")
                        nop.engine = inst.engine
                        nop.sync_info = mybir.SyncInfo(
                            on_wait=[w], on_update=[]
                        )
                        new_list.append(nop)
                    inst.sync_info = mybir.SyncInfo(
                        on_wait=waits[:limit], on_update=list(si.on_update)
                    )
                    changed = True
                new_list.append(inst)
            if changed:
                bb.instructions = new_list


def _build():
    if "nc" in _CACHE:
        return _CACHE["nc"]

    _install_tile_drain_patch()

    import concourse.bass as bass
    import concourse.mybir as mybir
    from concourse.tile import TileContext
    from concourse.masks import make_identity

    f32 = mybir.dt.float32
    f16 = mybir.dt.float16
    X = mybir.AxisListType.X

    nc = bass.Bass()

    # ---- DRAM I/O (identical on every core except qg/qd shards) ----
    fqF = nc.dram_tensor("fqF", [CF, BS, S], f16, kind="ExternalInput")
    fkF = nc.dram_tensor("fkF", [CF, BS, S], f16, kind="ExternalInput")
    d_qTF = nc.dram_tensor("d_qTF", [S, BS, DIM], f16, kind="ExternalInput")
    d_kF = nc.dram_tensor("d_kF", [DIM, BS, S], f16, kind="ExternalInput")
    g_qF = nc.dram_tensor("g_qF", [BS, DIM], f32, kind="ExternalInput")
    g_kF = nc.dram_tensor("g_kF", [BS, DIM], f32, kind="ExternalInput")
    g_qT5 = nc.dram_tensor("g_qT5", [DIM, BS], f16, kind="ExternalInput")
    qg = nc.dram_tensor("qg", [DIM, QS], f16, kind="ExternalInput")
    qd = nc.dram_tensor("qd", [DIM, QS], f16, kind="ExternalInput")

    out_d = nc.dram_tensor("out_d", [QS, BS, S], f16, kind="ExternalOutput")
    out_g = nc.dram_tensor("out_g", [BS, QS], f16, kind="ExternalOutput")
    out_pos = nc.dram_tensor("out_pos", [BS, 1 + S], f32, kind="ExternalOutput")

    fqF_r = fqF.rearrange("(t p) b s -> p t b s", p=128)   # [128, CT, BS, S]
    fkF_r = fkF.rearrange("(t p) b s -> p t b s", p=128)

    with TileContext(nc) as tc:
        with (
            tc.tile_pool(name="const", bufs=1) as const_pool,
            tc.tile_pool(name="queues", bufs=1) as queue_pool,
            tc.tile_pool(name="feat", bufs=1) as feat_pool,
            tc.tile_pool(name="dqm", bufs=1) as dqm_pool,
            tc.tile_pool(name="small", bufs=3) as small_pool,
            tc.tile_pool(name="stage", bufs=8) as stage_pool,
            tc.tile_pool(name="gstage", bufs=2) as gstage_pool,
        ):
            # ---- constants ----
            ident = const_pool.tile([128, 128], f32)
            make_identity(nc, ident)
            ident16 = const_pool.tile([128, 128], f16)
            nc.vector.tensor_copy(ident16[:], ident[:])
            ones = const_pool.tile([128, 1], f32)
            nc.vector.memset(ones, 1.0)

            # ---- loads, all on the sync HWDGE ring in priority order:
            # feature chunks first (they gate the cosine), then the small
            # tensors, then the big queue shards (needed only after the
            # match completes ~35 us in). ----
            fq_sb = feat_pool.tile([128, CT, BS, S], f16, tag="fq")
            fk_sb = feat_pool.tile([128, CT, BS, S], f16, tag="fk")
            for t in range(CT):
                nc.sync.dma_start(fq_sb[:, t], fqF_r[:, t, :, :])
                nc.sync.dma_start(fk_sb[:, t], fkF_r[:, t, :, :])

            d_qT_sb = const_pool.tile([128, BS, DIM], f16)   # padded K
            nc.vector.memset(d_qT_sb[:], 0.0)
            nc.sync.dma_start(d_qT_sb[:S, :, :], d_qTF[:, :, :])
            d_k_sb = const_pool.tile([128, BS, S], f16)
            nc.sync.dma_start(d_k_sb[:], d_kF[:, :, :])
            g_q_sb = const_pool.tile([BS, DIM], f32)
            nc.sync.dma_start(g_q_sb[:], g_qF[:, :])
            g_k_sb = const_pool.tile([BS, DIM], f32)
            nc.sync.dma_start(g_k_sb[:], g_kF[:, :])
            g_qT5_sb = const_pool.tile([128, BS], f16)
            nc.sync.dma_start(g_qT5_sb[:], g_qT5[:, :])

            qd_sb = queue_pool.tile([128, QS], f16, tag="qd")
            qg_sb = queue_pool.tile([128, QS], f16, tag="qg")
            for h in range(4):
                sl = slice(h * (QS // 4), (h + 1) * (QS // 4))
                nc.sync.dma_start(qd_sb[:, sl], qd[:, sl])
                nc.sync.dma_start(qg_sb[:, sl], qg[:, sl])

            posd_sb = const_pool.tile([S, BS], f32)          # pos_d [s, b]
            pos_sb = const_pool.tile([BS, 1 + S], f32)

            # ---- phase 1: match + gather, ALL 32 batches ----
            # 8 batches packed per PSUM bank: 4 column slots of 49 x 2
            # partition halves (tile_position col groups 0 / 64).
            dqm_all = dqm_pool.tile([128, BS * S], f16, tag="dqma")
            p1_psum = tc.tile_pool(name="p1psum", bufs=1, space="PSUM")
            pcos_pool = p1_psum.__enter__()
            poh_pool = pdqm_pool = ppos_pool = pcos_pool
            with nc.named_scope("p1"):
                pcos_t = [
                    pcos_pool.tile([128, 4 * S], f32, tag=f"pcos{p}",
                                   name=f"pcos{p}")
                    for p in range(BS // 8)
                ]
                # A start=True matmul clears the has_written bits of the
                # whole partition row of its PSUM bank (not just its own
                # columns), so only the FIRST column slot per (tile,
                # half) may carry start=True; the other slots' first
                # writes land on cleared bits and overwrite stale data
                # automatically.  batch b -> tile b//8, slot (b%8)//2,
                # half b%2; the b-ascending emit order guarantees slot 0
                # executes first per (tile, half).
                for t in range(CT):
                    for b in range(BS):
                        tile = pcos_t[b // 8]
                        slot = (b % 8) // 2
                        h = b % 2
                        s0 = slot * S
                        nc.tensor.matmul(
                            tile[64 * h : 64 * h + S, s0 : s0 + S],
                            fq_sb[:, t, b, :],
                            fk_sb[:, t, b, :],
                            start=(t == 0 and slot == 0),
                            stop=(t == CT - 1),
                            tile_position=(0, 64 * h),
                            skip_group_check=True,
                        )
                # argmax -> one-hot -> gather, one batch-pair at a time
                for bp in range(BS // 2):
                    tile = pcos_t[bp // 4]
                    s0 = (bp % 4) * S
                    csl = tile[:, s0 : s0 + S]           # 2 batches packed
                    cmax = small_pool.tile([128, 1], f32, tag="cmax")
                    nc.vector.reduce_max(out=cmax[:], in_=csl, axis=X)
                    onehT = small_pool.tile([128, S], f16, tag="onehT")
                    nc.vector.tensor_scalar(
                        onehT[:], csl, cmax[:], INV_TAU,
                        mybir.AluOpType.is_ge, mybir.AluOpType.mult,
                    )
                    poh = poh_pool.tile([S, 128], f16, tag="poh")
                    nc.tensor.transpose(poh, onehT[:], ident16[:])
                    for h in range(2):
                        bi = 2 * bp + h
                        oneh = small_pool.tile([128, S], f16, tag="oneh")
                        nc.vector.memset(oneh[:], 0.0)
                        nc.vector.tensor_copy(
                            oneh[:S, :], poh[:, 64 * h : 64 * h + S]
                        )
                        pdqm = pdqm_pool.tile([128, S], f32, tag="pdqm")
                        nc.tensor.matmul(
                            pdqm, d_qT_sb[:, bi, :], oneh[:],
                            start=True, stop=True,
                        )
                        nc.vector.tensor_copy(
                            dqm_all[:, bi * S : (bi + 1) * S], pdqm[:]
                        )

            # ---- pos logits for all batches (fills the match tail) ----
            with nc.named_scope("pos"):
                for bi in range(BS):
                    prod = small_pool.tile([128, S], f32, tag="prod")
                    nc.vector.tensor_tensor(
                        prod[:],
                        d_k_sb[:, bi, :],
                        dqm_all[:, bi * S : (bi + 1) * S],
                        mybir.AluOpType.mult,
                    )
                    ppos = ppos_pool.tile([S, 1], f32, tag="ppos")
                    nc.tensor.matmul(
                        ppos, prod[:], ones[:], start=True, stop=True
                    )
                    nc.vector.tensor_copy(posd_sb[:, bi : bi + 1], ppos[:])
                prodg = small_pool.tile([BS, DIM], f32, tag="prodg")
                nc.vector.tensor_tensor(
                    prodg[:], g_q_sb[:], g_k_sb[:], mybir.AluOpType.mult
                )
                posg = small_pool.tile([BS, 1], f32, tag="posg")
                nc.vector.reduce_sum(out=posg[:], in_=prodg[:], axis=X)
                nc.vector.tensor_scalar_mul(pos_sb[:, 0:1], posg[:], INV_TAU)
                pposT = ppos_pool.tile([BS, S], f32, tag="pposT")
                nc.tensor.transpose(pposT, posd_sb[:], ident[:S, :S])
                nc.vector.tensor_copy(pos_sb[:, 1:], pposT[:])
                nc.sync.dma_start(out_pos[:, :], pos_sb[:])

            p1_psum.__exit__(None, None, None)
            pmm_ctx = tc.tile_pool(name="pmm", bufs=6, space="PSUM")
            pmm_pool = pmm_ctx.__enter__()
            pg_ctx = tc.tile_pool(name="pg", bufs=2, space="PSUM")
            pg_pool = pg_ctx.__enter__()

            # ---- out_g = (g_q.T * invtau).T @ queue_g shard ----
            with nc.named_scope("gphase"), tc.high_priority():
                for nt4 in range(QS // 2048):
                    gst = gstage_pool.tile([BS, 4, 512], f16, tag="gstage")
                    for k in range(4):
                        nt = nt4 * 4 + k
                        pg = pg_pool.tile([BS, 512], f32, tag="pg")
                        nc.tensor.matmul(
                            pg,
                            g_qT5_sb[:],
                            qg_sb[:, nt * 512 : (nt + 1) * 512],
                            start=True,
                            stop=True,
                        )
                        nc.vector.tensor_copy(gst[:, k], pg[:])
                    nc.sync.dma_start(
                        out_g[:, nt4 * 2048 : (nt4 + 1) * 2048],
                        gst[:].rearrange("b k n -> b (k n)"),
                    )

            # ---- phase 2: out_d over the q shard, all 32 batches per tile ----
            with nc.named_scope("p2"):
                for qt in range(QT):
                    stg = stage_pool.tile([128, BS, S], f16, tag="stage")
                    for g in range(BG):
                        pmm = pmm_pool.tile([128, BPG * S], f32, tag="pmm")
                        nc.tensor.matmul(
                            pmm,
                            qd_sb[:, qt * 128 : (qt + 1) * 128],
                            dqm_all[:, g * BPG * S : (g + 1) * BPG * S],
                            start=True,
                            stop=True,
                        )
                        src = pmm[:].rearrange("p (b s) -> p b s", b=BPG)
                        dst = stg[:, g * BPG : (g + 1) * BPG, :]
                        if g % 2 == 0:
                            nc.vector.tensor_copy(dst, src)
                        else:
                            nc.scalar.copy(dst, src)
                    nc.sync.dma_start(
                        out_d[qt * 128 : (qt + 1) * 128, :, :], stg[:]
                    )
            pg_ctx.__exit__(None, None, None)
            pmm_ctx.__exit__(None, None, None)


    _split_multi_waits(nc, mybir)

    _CACHE["nc"] = nc
    return nc


def prepare_in_maps(inputs):
    g_q = np.ascontiguousarray(inputs["g_q"], dtype=np.float32)
    g_k = np.ascontiguousarray(inputs["g_k"], dtype=np.float32)
    d_q = np.asarray(inputs["d_q"], dtype=np.float32)
    d_k = np.asarray(inputs["d_k"], dtype=np.float32)
    feat_q = np.asarray(inputs["feat_q"], dtype=np.float32)
    feat_k = np.asarray(inputs["feat_k"], dtype=np.float32)
    queue_g = np.asarray(inputs["queue_g"], dtype=np.float32)
    queue_d = np.asarray(inputs["queue_d"], dtype=np.float32)

    def to_f16(a):
        # The PE mishandles fp16 subnormals in the weight path (NaN
        # products); flush them to zero (|err| <= 6.1e-5, negligible here).
        a = a.astype(np.float16)
        a[np.abs(a) < np.float16(6.104e-5)] = np.float16(0)
        return a

    fqX = to_f16(np.ascontiguousarray(feat_q.transpose(1, 0, 2)))  # [CF,BS,S]
    fkX = to_f16(np.ascontiguousarray(feat_k.transpose(1, 0, 2)))
    d_qT = to_f16(np.ascontiguousarray(d_q.transpose(2, 0, 1)))    # [S,BS,DIM]
    d_kX = to_f16(np.ascontiguousarray(d_k.transpose(1, 0, 2)))    # [DIM,BS,S]
    g_qT5 = to_f16(np.ascontiguousarray(g_q.T * np.float32(INV_TAU)))
    qg16 = to_f16(queue_g)
    qd16 = to_f16(queue_d)

    in_maps = []
    for c in range(NCORES):
        sh = slice(c * QS, (c + 1) * QS)
        in_maps.append(
            {
                "fqF": fqX,
                "fkF": fkX,
                "d_qTF": d_qT,
                "d_kF": d_kX,
                "g_qF": g_q,
                "g_kF": g_k,
                "g_qT5": g_qT5,
                "qg": np.ascontiguousarray(qg16[:, sh]),
                "qd": np.ascontiguousarray(qd16[:, sh]),
            }
        )
    return in_maps


def assemble(results) -> np.ndarray:
    BL = BS // NCORES
    out = np.empty((BS, 1 + Q, 1 + S), dtype=np.float32)
    for c in range(NCORES):
        out[c * BL : (c + 1) * BL, 0, :] = results[c]["out_pos"][
            c * BL : (c + 1) * BL
        ]
        rows = slice(1 + c * QS, 1 + (c + 1) * QS)
        out[:, rows, 0] = results[c]["out_g"].astype(np.float32)
        out[:, rows, 1:] = (
            results[c]["out_d"].transpose(1, 0, 2).astype(np.float32)
        )
    return out


def kernel(**inputs) -> np.ndarray:
    from concourse.bass_utils import run_bass_kernel_spmd

    nc = _build()
    in_maps = prepare_in_maps(inputs)
    res = run_bass_kernel_spmd(nc, in_maps, core_ids=list(range(NCORES)))
    return assemble(res.results)


# revision 11
# speedup vs baseline: 1.1802x; 1.1802x over previous
"""DenseCL contrastive-logits kernel for 8 Trainium2 NeuronCores.

Contract: kernel(**inputs) takes the FULL unsharded inputs (named as in
setup_inputs) and returns the full [32, 65537, 50] float32 output.

Sharding: the 65536-wide negative queues are split along the queue axis
across the 8 cores (8192 columns each); every other input is replicated.
There are NO collectives: profiling showed the runtime's cross-core sync
barrier + ncfw latency puts a ~70-95 us floor on any collective-gated
work, so instead EVERY core redundantly computes the match/gather stage
(cosine + argmax + d_q gather) for all 32 batches from fp16 features
(12.8 MB/core).  The feature DMA overlaps the cosine accumulation
matmuls chunk by chunk, then phase 2 (the 25.7 MB out_d stream) runs
DMA-bound with no cross-core dependency at all.

Precision: the match cosine runs in fp16.  Verified on the generated
inputs: the post-fp16-rounding top-2 margin of the cosine (0.0059 worst
case) is ~60x the fp32 accumulation noise, and the fp16 input rounding
is deterministic and identical between numpy and the PE, so the argmax
reproduces the reference's fp32 choice exactly.  The negative-logit
matmuls and outputs run in fp16 (values are O(50); ~4e-4 relative
error): single-PE-pass matmuls and half the output DMA bytes.  fp16
subnormals are flushed on the host (the PE weight path mishandles them).

PSUM has_written semantics (hardware-observed): a start=True matmul
clears the has_written bits of the whole partition row of its PSUM
bank, not just its own columns.  Wherever several accumulation groups
are packed into one bank at different column offsets, only the first
group's first matmul carries start=True; the other groups' first writes
then land on cleared bits and overwrite stale data automatically.

Math (per batch b, t = 1/tau = 5 folded into the one-hot):
  cosT[j, i] = sum_c feat_q[b, c, j] * feat_k[b, c, i]     (PE fp16,
               8 batches packed per PSUM bank: 4 col slots x 2 halves)
  onehotT[j, i] = t * (cosT[j, i] >= max_i cosT[j, :])      (DVE)
  onehot = onehotT^T                                        (PE transpose)
  d_qm5[d, j] = sum_i d_qT[b, i, d] * onehot[i, j]          (PE fp16, K=49)
  out_d[q, b, s] = sum_d queue_d[d, q] * d_qm5[b, d, s]     (PE fp16, q-shard)
  out_g[q, b]   = sum_d queue_g[d, q] * t * g_q[b, d]       (PE fp16, q-shard)
  pos_d[b, s]   = sum_d d_k[b, d, s] * d_qm5[b, d, s]       (all b, fused)
  pos_g[b]      = t * sum_d g_q[b, d] * g_k[b, d]           (all b)
"""

import numpy as np

BS, DIM, S, CF, Q = 32, 128, 49, 2048, 65536
NCORES = 8
QS = Q // NCORES          # 8192 queue columns per core
BG = 4                    # batch groups in the big matmul
BPG = BS // BG            # 8 batches per group (8*49 = 392 fp32 < 1 psum bank)
CT = CF // 128            # 16 contraction chunks for the cosine
QT = QS // 128            # 64 queue tiles per core
INV_TAU = 5.0

_CACHE = {}


def _install_tile_drain_patch():
    """walrus in this container rejects instructions with >1 sync wait
    ("Too many sync wait commands" in setupSyncWait).  TileContext's
    end-of-kernel drain carries one wait per semaphore used; split them
    across a chain of single-wait drain instructions (same engine, same
    semantics)."""
    import concourse.tile as tile_mod
    import concourse.mybir as mybir
    from concourse.vector_clock import ScopedClock

    if getattr(tile_mod.TileContext, "_drain_patch_installed", False):
        return

    def _drain_and_barrier(self, tick_clock, wait_clock):
        nc = self.nc
        drain_inst = nc.sync.drain()
        wait_clock.add_sem_waits(
            drain_inst.ins, ScopedClock({None: tick_clock.global_clock})
        )
        waits = list(drain_inst.ins.sync_info.on_wait)
        if len(waits) > 1:
            drain_inst.ins.sync_info = mybir.SyncInfo(
                on_wait=waits[:1], on_update=[]
            )
            for i in range(1, len(waits)):
                extra = nc.sync.drain()
                extra.ins.sync_info = mybir.SyncInfo(
                    on_wait=waits[i : i + 1], on_update=[]
                )
        nc.all_engine_barrier()
        assert self.sems is not None
        popped = nc._tile_sem_poison_stack.pop()
        assert popped is self._sem_poison
        nc.clear_and_free_semaphores(list(self.sems.allocated().values()))
        nc.all_engine_barrier()

    tile_mod.TileContext._drain_and_barrier = _drain_and_barrier
    tile_mod.TileContext._drain_patch_installed = True


def _split_multi_waits(nc, mybir, limit=1):
    """walrus codegen here rejects instructions with more than one sync
    wait.  Hoist excess waits onto InstNoOp carriers inserted immediately
    before the offender in the same block (same engine stream => same
    semantics: all waits still execute before the instruction)."""
    n_new = 0
    for f in nc.m.functions:
        for bb in f.blocks:
            new_list = []
            changed = False
            for inst in bb.instructions:
                si = inst.sync_info
                waits = list(si.on_wait) if si is not None else []
                if len(waits) > limit:
                    for w in waits[limit:]:
                        n_new += 1
                        nop = mybir.InstNoOp(name=f"WS-{n_new}")
                        nop.engine = inst.engine
                        nop.sync_info = mybir.SyncInfo(
                            on_wait=[w], on_update=[]
                        )
                        new_list.append(nop)
                    inst.sync_info = mybir.SyncInfo(
                        on_wait=waits[:limit], on_update=list(si.on_update)
                    )
                    changed = True
                new_list.append(inst)
            if changed:
                bb.instructions = new_list


def _build():
    if "nc" in _CACHE:
        return _CACHE["nc"]

    _install_tile_drain_patch()

    import concourse.bass as bass
    import concourse.mybir as mybir
    from concourse.tile import TileContext
    from concourse.masks import make_identity

    f32 = mybir.dt.float32
    f16 = mybir.dt.float16
    X = mybir.AxisListType.X

    nc = bass.Bass()

    # ---- DRAM I/O (identical on every core except qg/qd shards) ----
    fqF = nc.dram_tensor("fqF", [CF, BS, S], f16, kind="ExternalInput")
    fkF = nc.dram_tensor("fkF", [CF, BS, S], f16, kind="ExternalInput")
    d_qTF = nc.dram_tensor("d_qTF", [S, BS, DIM], f16, kind="ExternalInput")
    d_kF = nc.dram_tensor("d_kF", [DIM, BS, S], f16, kind="ExternalInput")
    g_qF = nc.dram_tensor("g_qF", [BS, DIM], f32, kind="ExternalInput")
    g_kF = nc.dram_tensor("g_kF", [BS, DIM], f32, kind="ExternalInput")
    g_qT5 = nc.dram_tensor("g_qT5", [DIM, BS], f16, kind="ExternalInput")
    qg = nc.dram_tensor("qg", [DIM, QS], f16, kind="ExternalInput")
    qd = nc.dram_tensor("qd", [DIM, QS], f16, kind="ExternalInput")

    out_d = nc.dram_tensor("out_d", [QS, BS, S], f16, kind="ExternalOutput")
    out_g = nc.dram_tensor("out_g", [QS, BS], f16, kind="ExternalOutput")
    out_pos = nc.dram_tensor("out_pos", [1, BS * S], f32, kind="ExternalOutput")
    out_posg = nc.dram_tensor("out_posg", [BS, 1], f32, kind="ExternalOutput")

    fqF_r = fqF.rearrange("(t p) b s -> p t b s", p=128)   # [128, CT, BS, S]
    fkF_r = fkF.rearrange("(t p) b s -> p t b s", p=128)
    out_g_r = out_g.rearrange("(w t p) b -> p w t b", p=128, t=16)

    with TileContext(nc) as tc:
        with (
            tc.tile_pool(name="const", bufs=1) as const_pool,
            tc.tile_pool(name="queues", bufs=1) as queue_pool,
            tc.tile_pool(name="feat", bufs=1) as feat_pool,
            tc.tile_pool(name="dqm", bufs=1) as dqm_pool,
            tc.tile_pool(name="small", bufs=3) as small_pool,
            tc.tile_pool(name="posp", bufs=1) as pos_pool,
            tc.tile_pool(name="stage", bufs=8) as stage_pool,
            tc.tile_pool(name="gstage", bufs=2) as gstage_pool,
        ):
            # ---- constants ----
            ident = const_pool.tile([128, 128], f32)
            make_identity(nc, ident)
            ident16 = const_pool.tile([128, 128], f16)
            nc.vector.tensor_copy(ident16[:], ident[:])
            ones = const_pool.tile([128, 1], f32)
            nc.vector.memset(ones, 1.0)

            # ---- loads, all on the sync HWDGE ring in priority order:
            # feature chunks first (they gate the cosine), then the small
            # tensors, then qd (gates phase 2), then qg. ----
            fq_sb = feat_pool.tile([128, CT, BS, S], f16, tag="fq")
            fk_sb = feat_pool.tile([128, CT, BS, S], f16, tag="fk")
            for t in range(CT):
                nc.sync.dma_start(fq_sb[:, t], fqF_r[:, t, :, :])
                nc.sync.dma_start(fk_sb[:, t], fkF_r[:, t, :, :])

            d_qT_sb = const_pool.tile([S, BS, DIM], f16)
            nc.sync.dma_start(d_qT_sb[:], d_qTF[:, :, :])
            d_k_sb = const_pool.tile([128, BS, S], f16)
            nc.sync.dma_start(d_k_sb[:], d_kF[:, :, :])
            g_q_sb = const_pool.tile([BS, DIM], f32)
            nc.sync.dma_start(g_q_sb[:], g_qF[:, :])
            g_k_sb = const_pool.tile([BS, DIM], f32)
            nc.sync.dma_start(g_k_sb[:], g_kF[:, :])
            g_qT5_sb = const_pool.tile([128, BS], f16)
            nc.sync.dma_start(g_qT5_sb[:], g_qT5[:, :])

            qd_sb = queue_pool.tile([128, QS], f16, tag="qd")
            qg_sb = queue_pool.tile([128, QS], f16, tag="qg")
            for h in range(4):
                sl = slice(h * (QS // 4), (h + 1) * (QS // 4))
                nc.sync.dma_start(qd_sb[:, sl], qd[:, sl])
            for h in range(4):
                sl = slice(h * (QS // 4), (h + 1) * (QS // 4))
                nc.sync.dma_start(qg_sb[:, sl], qg[:, sl])

            # ---- phase 1: match + gather, ALL 32 batches ----
            # 8 batches packed per PSUM bank: 4 column slots of 49 x 2
            # partition halves (tile_position col groups 0 / 64).
            dqm_all = dqm_pool.tile([128, BS * S], f16, tag="dqma")
            p1_psum = tc.tile_pool(name="p1psum", bufs=1, space="PSUM")
            pcos_pool = p1_psum.__enter__()
            poh_pool = pdqm_pool = ppos_pool = pcos_pool
            with nc.named_scope("p1"):
                pcos_t = [
                    pcos_pool.tile([128, 4 * S], f32, tag=f"pcos{p}",
                                   name=f"pcos{p}")
                    for p in range(BS // 8)
                ]
                # batch b -> tile b//8, col slot (b%8)//2, half b%2; the
                # b-ascending emit order guarantees slot 0 executes first
                # per (tile, half), so only it carries start=True.
                for t in range(CT):
                    for b in range(BS):
                        tile = pcos_t[b // 8]
                        slot = (b % 8) // 2
                        h = b % 2
                        s0 = slot * S
                        nc.tensor.matmul(
                            tile[64 * h : 64 * h + S, s0 : s0 + S],
                            fq_sb[:, t, b, :],
                            fk_sb[:, t, b, :],
                            start=(t == 0 and slot == 0),
                            stop=(t == CT - 1),
                            tile_position=(0, 64 * h),
                            skip_group_check=True,
                        )
                # argmax -> one-hot -> gather, one batch-pair at a time
                for bp in range(BS // 2):
                    tile = pcos_t[bp // 4]
                    s0 = (bp % 4) * S
                    csl = tile[:, s0 : s0 + S]           # 2 batches packed
                    cmax = small_pool.tile([128, 1], f32, tag="cmax")
                    nc.vector.reduce_max(out=cmax[:], in_=csl, axis=X)
                    onehT = small_pool.tile([128, S], f16, tag="onehT")
                    nc.vector.tensor_scalar(
                        onehT[:], csl, cmax[:], INV_TAU,
                        mybir.AluOpType.is_ge, mybir.AluOpType.mult,
                    )
                    poh = poh_pool.tile([S, 128], f16, tag="poh")
                    nc.tensor.transpose(poh, onehT[:], ident16[:])
                    for h in range(2):
                        bi = 2 * bp + h
                        oneh = small_pool.tile([S, S], f16, tag="oneh")
                        nc.vector.tensor_copy(
                            oneh[:], poh[:, 64 * h : 64 * h + S]
                        )
                        pdqm = pdqm_pool.tile([128, S], f32, tag="pdqm")
                        nc.tensor.matmul(
                            pdqm, d_qT_sb[:, bi, :], oneh[:],
                            start=True, stop=True,
                        )
                        nc.scalar.copy(
                            dqm_all[:, bi * S : (bi + 1) * S], pdqm[:]
                        )

            # ---- pos logits, fused over all batches ----
            with nc.named_scope("pos"):
                prod = pos_pool.tile([128, BS * S], f32, tag="prod")
                nc.vector.tensor_tensor(
                    prod[:],
                    d_k_sb[:].rearrange("p b s -> p (b s)"),
                    dqm_all[:],
                    mybir.AluOpType.mult,
                )
                posrow = pos_pool.tile([1, BS * S], f32, tag="posrow")
                for i in range(BG):
                    sl = slice(i * BPG * S, (i + 1) * BPG * S)
                    ppos = ppos_pool.tile([1, BPG * S], f32, tag="ppos")
                    nc.tensor.matmul(
                        ppos, ones[:, :], prod[:, sl], start=True, stop=True
                    )
                    nc.scalar.copy(posrow[:, sl], ppos[:])
                nc.sync.dma_start(out_pos[:, :], posrow[:])
                prodg = small_pool.tile([BS, DIM], f32, tag="prodg")
                nc.vector.tensor_tensor(
                    prodg[:], g_q_sb[:], g_k_sb[:], mybir.AluOpType.mult
                )
                posg = small_pool.tile([BS, 1], f32, tag="posg")
                nc.vector.reduce_sum(out=posg[:], in_=prodg[:], axis=X)
                posg5 = small_pool.tile([BS, 1], f32, tag="posg5")
                nc.vector.tensor_scalar_mul(posg5[:], posg[:], INV_TAU)
                nc.sync.dma_start(out_posg[:, :], posg5[:])

            p1_psum.__exit__(None, None, None)
            pmm_ctx = tc.tile_pool(name="pmm", bufs=3, space="PSUM")
            pmm_pool = pmm_ctx.__enter__()
            pg_ctx = tc.tile_pool(name="pg", bufs=2, space="PSUM")
            pg_pool = pg_ctx.__enter__()

            # ---- out_g[q, b] = qg^T (g_q * invtau): q-major so the PSUM
            # -> SBUF copies use all 128 partitions; 4 q-tiles packed per
            # PSUM bank. ----
            with nc.named_scope("gphase"), tc.high_priority():
                for w in range(4):
                    gst = gstage_pool.tile([128, 16, BS], f16, tag="gstage")
                    for j4 in range(4):
                        pgt = pg_pool.tile([128, 4, BS], f32, tag="pg")
                        for k in range(4):
                            nt = w * 16 + j4 * 4 + k
                            nc.tensor.matmul(
                                pgt[:, k, :],
                                qg_sb[:, nt * 128 : (nt + 1) * 128],
                                g_qT5_sb[:],
                                start=(k == 0),
                                stop=True,
                                skip_group_check=True,
                            )
                        nc.vector.tensor_copy(
                            gst[:, 4 * j4 : 4 * j4 + 4, :], pgt[:]
                        )
                    nc.sync.dma_start(out_g_r[:, w, :, :], gst[:])

            # ---- phase 2: out_d over the q shard, all 32 batches per
            # tile; two matmuls share a two-bank PSUM tile so each half
            # needs only one fused PSUM->SBUF copy. ----
            with nc.named_scope("p2"):
                for qt in range(QT):
                    stg = stage_pool.tile([128, BS, S], f16, tag="stage")
                    for half in range(2):
                        pmm = pmm_pool.tile([128, 2, 512], f32, tag="pmm")
                        for g2 in range(2):
                            g = 2 * half + g2
                            nc.tensor.matmul(
                                pmm[:, g2, : BPG * S],
                                qd_sb[:, qt * 128 : (qt + 1) * 128],
                                dqm_all[:, g * BPG * S : (g + 1) * BPG * S],
                                start=True,
                                stop=True,
                            )
                        src = pmm[:, :, : BPG * S].rearrange(
                            "p c (b s) -> p c b s", b=BPG
                        )
                        dst = stg[
                            :, half * 2 * BPG : (half + 1) * 2 * BPG, :
                        ].rearrange("p (c b) s -> p c b s", c=2)
                        if half == 0:
                            nc.vector.tensor_copy(dst, src)
                        else:
                            nc.scalar.copy(dst, src)
                    nc.sync.dma_start(
                        out_d[qt * 128 : (qt + 1) * 128, :, :], stg[:]
                    )
            pg_ctx.__exit__(None, None, None)
            pmm_ctx.__exit__(None, None, None)


    _split_multi_waits(nc, mybir)

    _CACHE["nc"] = nc
    return nc


def prepare_in_maps(inputs):
    g_q = np.ascontiguousarray(inputs["g_q"], dtype=np.float32)
    g_k = np.ascontiguousarray(inputs["g_k"], dtype=np.float32)
    d_q = np.asarray(inputs["d_q"], dtype=np.float32)
    d_k = np.asarray(inputs["d_k"], dtype=np.float32)
    feat_q = np.asarray(inputs["feat_q"], dtype=np.float32)
    feat_k = np.asarray(inputs["feat_k"], dtype=np.float32)
    queue_g = np.asarray(inputs["queue_g"], dtype=np.float32)
    queue_d = np.asarray(inputs["queue_d"], dtype=np.float32)

    def to_f16(a):
        # The PE mishandles fp16 subnormals in the weight path (NaN
        # products); flush them to zero (|err| <= 6.1e-5, negligible here).
        a = a.astype(np.float16)
        a[np.abs(a) < np.float16(6.104e-5)] = np.float16(0)
        return a

    fqX = to_f16(np.ascontiguousarray(feat_q.transpose(1, 0, 2)))  # [CF,BS,S]
    fkX = to_f16(np.ascontiguousarray(feat_k.transpose(1, 0, 2)))
    d_qT = to_f16(np.ascontiguousarray(d_q.transpose(2, 0, 1)))    # [S,BS,DIM]
    d_kX = to_f16(np.ascontiguousarray(d_k.transpose(1, 0, 2)))    # [DIM,BS,S]
    g_qT5 = to_f16(np.ascontiguousarray(g_q.T * np.float32(INV_TAU)))
    qg16 = to_f16(queue_g)
    qd16 = to_f16(queue_d)

    in_maps = []
    for c in range(NCORES):
        sh = slice(c * QS, (c + 1) * QS)
        in_maps.append(
            {
                "fqF": fqX,
                "fkF": fkX,
                "d_qTF": d_qT,
                "d_kF": d_kX,
                "g_qF": g_q,
                "g_kF": g_k,
                "g_qT5": g_qT5,
                "qg": np.ascontiguousarray(qg16[:, sh]),
                "qd": np.ascontiguousarray(qd16[:, sh]),
            }
        )
    return in_maps


def assemble(results) -> np.ndarray:
    BL = BS // NCORES
    out = np.empty((BS, 1 + Q, 1 + S), dtype=np.float32)
    for c in range(NCORES):
        posd = results[c]["out_pos"].reshape(BS, S)
        posg = results[c]["out_posg"].reshape(BS)
        bl = slice(c * BL, (c + 1) * BL)
        out[bl, 0, 1:] = posd[bl]
        out[bl, 0, 0] = posg[bl]
        rows = slice(1 + c * QS, 1 + (c + 1) * QS)
        out[:, rows, 0] = results[c]["out_g"].T.astype(np.float32)
        out[:, rows, 1:] = (
            results[c]["out_d"].transpose(1, 0, 2).astype(np.float32)
        )
    return out


def kernel(**inputs) -> np.ndarray:
    from concourse.bass_utils import run_bass_kernel_spmd

    nc = _build()
    in_maps = prepare_in_maps(inputs)
    res = run_bass_kernel_spmd(nc, in_maps, core_ids=list(range(NCORES)))
    return assemble(res.results)
